# revision 6
# baseline (speedup 1.0000x reference)
"""Trainium2 Bass kernel for the Event-SNN MLP forward pass.

Model (see reference): T timesteps; per step Bernoulli input spikes
x_t = (input > u_t) with u_t ~ U(0,1) from jax threefry key(42); membrane
h1 += x_t @ w1.T, spike s1 = h1 > 0.5, reset + decay 0.2; h2 += s1 @ w2.T,
spike s2, reset + decay; output = mean_t s2.

Strategy: pure data parallelism over the batch (8192 -> 8 cores x 1024).
The Bernoulli draws u_t depend only on key(42) (not on data), so the spike
tensors are computed bit-exactly on host CPU with jax and shipped to the
device as bf16 {0,1} in a matmul-ready layout. On device, each timestep is
two matmul groups (x @ w1T, s1 @ w2T) + fused vector ops for the membrane
update, with all floating-point rounding steps matching the reference's
order exactly. fp32 weights are split into bf16 hi+lo parts (x and s1 are
binary, hence exact in bf16), so each product is exact and PSUM accumulates
in fp32: the result matches a CPU fp32 matmul to ~1e-7, which measured
zero spike flips vs the fp64 reference.

Layout per core (batch B=1024):
  xspk [T, 112, 7, B]  bf16   partition p, k-chunk c: input feature = c*112+p
  w1h/w1l [112, 7, 400] bf16  w1T hi/lo chunks (same feature mapping)
  w2h/w2l [100, 4, 10]  bf16  w2T chunks: hidden = cm*100 + p
  out  [10, B] f32            spike counts (acc); host divides by T

Per timestep on device:
  mm1: for (bt in 2 batch halves, cm in 4 hidden chunks):
       psum[100,512] = sum_{hi,lo} sum_{7 k-chunks} w1T_chunk.T @ x_chunk
  A:   h1 = carry*0.2 + psum          (DVE scalar_tensor_tensor)
  B:   s1 = (h1 > 0.5) -> bf16        (DVE tensor_scalar, 2x perf mode)
  C:   carry = (h1 <= 0.5) * h1       (DVE)
  mm2: psum2[10,512/bank] = sum_{hi,lo} sum_{4 chunks} w2T_chunk.T @ s1_chunk
       (emitted one step late so the PE never stalls on s1)
  A2:  h2 = carry2*0.2 + psum2; ACC: acc += (h2 > 0.5); C2: carry2 = (h2<=0.5)*h2

NB: GpSimd is deliberately unused — its tensor_scalar measures ~32us per
[100,2048] op on HW and its SBUF port lock stalls concurrent DVE TT ops.
"""

import os
import numpy as np
import ml_dtypes

N_CORES = 8
B_TOTAL = 8192
IN_F = 784
HID = 400
OUT_F = 10
KP, KC = 112, 7          # k (input-feature) partition tiling: 7 chunks x 112
MP, MC = 100, 4          # hidden chunk tiling: 4 chunks x 100
BT_W = 512               # batch tile width (one PSUM bank of fp32)

B = B_TOTAL // N_CORES   # 1024 per core
NBT = B // BT_W          # 2 batch tiles per core

BF16 = ml_dtypes.bfloat16

_compiled = {}           # (T, strategy) -> nc


def _build_bass(T: int, strategy: str):
    import concourse.mybir as mybir
    from concourse.tile import TileContext
    from concourse import bacc

    f32 = mybir.dt.float32
    bf16 = mybir.dt.bfloat16
    mm_dt = {"bf16x2": bf16, "f32r": mybir.dt.float32r, "f32": f32}[strategy]
    x_dt = mm_dt
    n_pass = 2 if strategy == "bf16x2" else 1
    ALU = mybir.AluOpType

    nc = bacc.Bacc("TRN2", target_bir_lowering=False, debug=False, num_devices=N_CORES)

    xspk = nc.declare_dram_parameter("xspk", [T, KP, KC, B], x_dt, isOutput=False)
    w1_d = [nc.declare_dram_parameter(n, [KP, KC, HID], mm_dt, isOutput=False)
            for n in (["w1h", "w1l"] if n_pass == 2 else ["w1h"])]
    w2_d = [nc.declare_dram_parameter(n, [MP, MC, OUT_F], mm_dt, isOutput=False)
            for n in (["w2h", "w2l"] if n_pass == 2 else ["w2h"])]
    out_d = nc.declare_dram_parameter("out", [OUT_F, B], f32, isOutput=True)

    with TileContext(nc) as tc:
        with (
            tc.tile_pool(name="weights", bufs=1) as wpool,
            tc.tile_pool(name="state", bufs=1) as spool,
            tc.tile_pool(name="xin", bufs=3) as xpool,
            tc.tile_pool(name="s1p", bufs=2) as s1pool,
            tc.tile_pool(name="ps1", bufs=6, space="PSUM") as ps1pool,
            tc.tile_pool(name="ps2", bufs=1, space="PSUM") as ps2pool,
        ):
            w1_t = []
            for i, d in enumerate(w1_d):
                w = wpool.tile([KP, KC, HID], mm_dt, tag=f"w1_{i}")
                nc.sync.dma_start(out=w[:], in_=d[:])
                w1_t.append(w)
            w2_t = []
            for i, d in enumerate(w2_d):
                w = wpool.tile([MP, MC, OUT_F], mm_dt, tag=f"w2_{i}")
                nc.sync.dma_start(out=w[:], in_=d[:])
                w2_t.append(w)

            h1 = spool.tile([MP, MC, B], f32, tag="h1")
            carry = spool.tile([MP, MC, B], f32, tag="carry")
            h2 = spool.tile([OUT_F, B], f32, tag="h2")
            carry2 = spool.tile([OUT_F, B], f32, tag="carry2")
            acc = spool.tile([OUT_F, B], f32, tag="acc")
            nc.vector.memset(carry[:], 0.0)
            nc.vector.memset(carry2[:], 0.0)
            nc.vector.memset(acc[:], 0.0)

            def bts(bt):
                return slice(bt * BT_W, (bt + 1) * BT_W)

            pend = None  # (s1_tile, t) whose mm2 + h2 chain is not yet emitted

            def emit_mm2_and_h2(s1_t):
                # one psum tile for all batch tiles: each matmul group targets
                # its own 2KB bank-aligned [10, 512] slice
                ps2 = ps2pool.tile([OUT_F, B], f32, tag="ps2")
                for bt in range(NBT):
                    n_mm = n_pass * MC
                    i = 0
                    for w in w2_t:
                        for cm in range(MC):
                            nc.tensor.matmul(
                                ps2[:, bts(bt)], lhsT=w[:, cm, :],
                                rhs=s1_t[:, cm, bts(bt)],
                                start=(i == 0), stop=(i == n_mm - 1))
                            i += 1
                nc.vector.scalar_tensor_tensor(
                    out=h2[:], in0=carry2[:], scalar=0.2,
                    in1=ps2[:], op0=ALU.mult, op1=ALU.add)
                nc.vector.scalar_tensor_tensor(
                    out=acc[:], in0=h2[:], scalar=0.5,
                    in1=acc[:], op0=ALU.is_gt, op1=ALU.add)
                nc.vector.scalar_tensor_tensor(
                    out=carry2[:], in0=h2[:], scalar=0.5,
                    in1=h2[:], op0=ALU.is_le, op1=ALU.mult)

            for t in range(T):
                xt = xpool.tile([KP, KC, B], x_dt, tag="xt")
                nc.sync.dma_start(out=xt[:], in_=xspk[t])

                # mm1 for all batch tiles / hidden chunks of this step
                ps_tiles = {}
                for bt in range(NBT):
                    for cm in range(MC):
                        ps = ps1pool.tile([MP, BT_W], f32, tag="ps1")
                        n_mm = n_pass * KC
                        i = 0
                        for w in w1_t:
                            for kc in range(KC):
                                nc.tensor.matmul(
                                    ps[:], lhsT=w[:, kc, cm * MP:(cm + 1) * MP],
                                    rhs=xt[:, kc, bts(bt)],
                                    start=(i == 0), stop=(i == n_mm - 1))
                                i += 1
                        ps_tiles[(bt, cm)] = ps

                # h1 membrane update + spikes
                s1_t = s1pool.tile([MP, MC, B], mm_dt, tag="s1")
                for bt in range(NBT):
                    for cm in range(MC):
                        nc.vector.scalar_tensor_tensor(
                            out=h1[:, cm, bts(bt)], in0=carry[:, cm, bts(bt)],
                            scalar=0.2, in1=ps_tiles[(bt, cm)][:],
                            op0=ALU.mult, op1=ALU.add)
                    nc.vector.tensor_scalar(
                        out=s1_t[:, :, bts(bt)], in0=h1[:, :, bts(bt)],
                        scalar1=0.5, scalar2=None, op0=ALU.is_gt)
                    nc.vector.scalar_tensor_tensor(
                        out=carry[:, :, bts(bt)], in0=h1[:, :, bts(bt)], scalar=0.5,
                        in1=h1[:, :, bts(bt)], op0=ALU.is_le, op1=ALU.mult)

                # second layer for the PREVIOUS step (keeps PE stall-free)
                if pend is not None:
                    emit_mm2_and_h2(pend)
                pend = s1_t

            if pend is not None:
                emit_mm2_and_h2(pend)

            nc.sync.dma_start(out=out_d[:], in_=acc[:])

    nc.compile()
    return nc


def _get_nc(T: int, strategy: str):
    key = (T, strategy)
    if key not in _compiled:
        _compiled[key] = _build_bass(T, strategy)
    return _compiled[key]


def _host_pack(input_arr: np.ndarray, w1: np.ndarray, w2: np.ndarray, T: int,
               strategy: str):
    """Host-side: bit-exact Bernoulli spikes via jax threefry (CPU), packed
    into the per-core device layouts."""
    import jax
    import jax.numpy as jnp

    cpu = jax.devices("cpu")[0]
    x_np_dt = BF16 if strategy == "bf16x2" else np.float32

    with jax.default_device(cpu):
        inp = jnp.asarray(np.asarray(input_arr, np.float32))
        keys = jax.random.split(jax.random.key(42), T)

        @jax.jit
        def spikes(key):
            u = jax.random.uniform(key, inp.shape, dtype=inp.dtype)
            x = (inp > u)                              # [B_TOTAL, IN_F] bool
            xt = x.T.reshape(KC, KP, B_TOTAL).transpose(1, 0, 2)  # [112,7,B_TOTAL]
            return xt.astype(jnp.bfloat16 if strategy == "bf16x2" else jnp.float32)

        X = np.empty((T, KP, KC, B_TOTAL), x_np_dt)
        for t in range(T):
            X[t] = np.asarray(spikes(keys[t]))

    def split(w):  # fp32 -> (hi, lo) bf16 with w ~= hi + lo
        hi = w.astype(BF16)
        lo = (w - hi.astype(np.float32)).astype(BF16)
        return hi, lo

    w1T = np.ascontiguousarray(np.asarray(w1, np.float32).T)   # [784, 400]
    w2T = np.ascontiguousarray(np.asarray(w2, np.float32).T)   # [400, 10]
    w1P = w1T.reshape(KC, KP, HID).transpose(1, 0, 2)          # [112, 7, 400]
    w2P = w2T.reshape(MC, MP, OUT_F).transpose(1, 0, 2)        # [100, 4, 10]

    in_maps = []
    for c in range(N_CORES):
        m = {"xspk": np.ascontiguousarray(X[:, :, :, c * B:(c + 1) * B])}
        if strategy == "bf16x2":
            m["w1h"], m["w1l"] = (np.ascontiguousarray(a) for a in split(w1P))
            m["w2h"], m["w2l"] = (np.ascontiguousarray(a) for a in split(w2P))
        else:
            m["w1h"] = np.ascontiguousarray(w1P)
            m["w2h"] = np.ascontiguousarray(w2P)
        in_maps.append(m)
    return in_maps


def _run(inputs: dict, strategy: str = None, trace: bool = False, tmpdir=None):
    from concourse.bass_utils import run_bass_kernel_spmd

    strategy = strategy or os.environ.get("SNN_STRATEGY", "bf16x2")
    T = int(inputs["time_window"])
    inp = np.asarray(inputs["input"], np.float32)
    assert inp.shape == (B_TOTAL, IN_F), inp.shape

    nc = _get_nc(T, strategy)
    in_maps = _host_pack(inp, inputs["w1"], inputs["w2"], T, strategy)
    res = run_bass_kernel_spmd(nc, in_maps, list(range(N_CORES)),
                               trace=trace, tmpdir=tmpdir)

    out = np.empty((B_TOTAL, OUT_F), np.float32)
    for c in range(N_CORES):
        out[c * B:(c + 1) * B, :] = res.results[c]["out"].T
    out /= np.float32(T)
    return out, res


def kernel(**inputs) -> np.ndarray:
    out, _ = _run(inputs)
    return out
